# revision 1
# baseline (speedup 1.0000x reference)
"""Trainium2 Bass kernel for nn_ExtractorMLP (GNN edge cosine-similarity).

Math:  out[e] = cos_sim(mlp(emb[col[e]]), mlp(emb[row[e]]))
where  mlp(x) = elu(x @ W1.T + b1) @ W2.T + b2   (b1 = b2 = 0 for this problem)

Strategy (per the edge-data-parallel sharding hint):
  * Phase 1 (per node, replicated on every core): compute the normalized MLP
    output table  t[v] = g[v] / max(||g[v]||, eps)  for all N nodes, assembled
    directly in SBUF in the packed layout dma_gather expects (node v lives in
    partition v%128 at free-byte offset (v//128)*256, bf16).
  * Phase 2 (edges, sharded 8 ways): each core gathers t[col], t[row] for its
    edge slice with SBUF-source transpose dma_gather (output lands as
    [feature=128 partitions, edge columns]), multiplies elementwise on DVE and
    contracts over features with a ones-vector matmul on the tensor engine.
    dma_gather indices are int16, so edges are bucketed on the host by
    (col < 32768, row < 32768) and each bucket gathers from the matching
    half of the table with half-local indices.

ELU identity used on device:  elu(x) = max(min(exp(x), 1) - 1, x)
"""

import math

import numpy as np
import ml_dtypes

BF16 = ml_dtypes.bfloat16

H = 128          # feature dim
P = 128          # partitions
CHUNK = 512      # edges per reduce-matmul
GT = 4096        # edges per dma_gather instruction
HALF = 32768     # int16 index limit: table split point
NCORES = 8
NSWQ = 1      # SWDGE queues: >1 corrupts concurrent gathers (racy ucode)
ST_W = 512       # phase-1 supertile width (nodes)

_PROG_CACHE: dict = {}
LAST_RESULTS = None  # test harness can inspect exec_time_ns


def _build_program(n_pad, half, nck, trace_label=""):
    """Build the (shared, SPMD) bass program.

    n_pad: padded node count (multiple of 128) = table free-dim elems/partition
    half:  table split element offset (multiple of 128)
    nck:   per-bucket chunk counts (len 4), shared across cores
    """
    import concourse.bacc as bacc
    import concourse.mybir as mybir
    import concourse.tile as tile
    from concourse import library_config
    from contextlib import ExitStack

    f32 = mybir.dt.float32
    bf16 = mybir.dt.bfloat16
    i16 = mybir.dt.int16
    Alu = mybir.AluOpType
    Act = mybir.ActivationFunctionType

    n_chunks = sum(nck)
    n_groups = math.ceil(n_chunks / P)
    stream_cols = n_chunks * (CHUNK // 16)   # wrapped-idx columns
    n_blocks = n_pad // H

    nc = bacc.Bacc("TRN2", target_bir_lowering=False, debug=False,
                   num_devices=NCORES, num_swdge_queues=NSWQ)

    embT = nc.dram_tensor("embT", [P, n_pad], bf16, kind="ExternalInput")
    w1t_d = nc.dram_tensor("w1t", [H, H], bf16, kind="ExternalInput")
    w2t_d = nc.dram_tensor("w2t", [H, H], bf16, kind="ExternalInput")
    cidx_d = nc.dram_tensor("cidx", [P, stream_cols], i16, kind="ExternalInput")
    ridx_d = nc.dram_tensor("ridx", [P, stream_cols], i16, kind="ExternalInput")
    out_d = nc.dram_tensor("out", [n_groups, P, CHUNK], f32, kind="ExternalOutput")

    with ExitStack() as ctx:
        tc = ctx.enter_context(tile.TileContext(nc))
        const = ctx.enter_context(tc.tile_pool(name="const", bufs=1))
        p1 = ctx.enter_context(tc.tile_pool(name="p1", bufs=3))
        p2 = ctx.enter_context(tc.tile_pool(name="p2", bufs=2))
        pprod = ctx.enter_context(tc.tile_pool(name="pprod", bufs=4))
        ps1 = ctx.enter_context(tc.tile_pool(name="ps1", bufs=2, space="PSUM"))
        ps2 = ctx.enter_context(tc.tile_pool(name="ps2", bufs=2, space="PSUM"))
        pso = ctx.enter_context(tc.tile_pool(name="pso", bufs=2, space="PSUM"))

        nc.gpsimd.load_library(library_config.mlp)

        # --- constants / persistent tiles ---
        table = const.tile([P, n_pad], bf16, tag="table")
        w1t = const.tile([H, H], bf16, tag="w1t")
        w2t = const.tile([H, H], bf16, tag="w2t")
        # sliding one-hot: onehot[:, 127-p : 255-p] has ones in column p only;
        # used as lhsT so chunk p's dot-row lands in PSUM partition p.
        onehot = const.tile([P, 2 * P - 1], bf16, tag="onehot")
        ss_all = const.tile([P, n_blocks], f32, tag="ss_all")
        r_all = const.tile([P, n_blocks], f32, tag="r_all")
        cidx = const.tile([P, stream_cols], i16, tag="cidx")
        ridx = const.tile([P, stream_cols], i16, tag="ridx")
        nc.sync.dma_start(out=w1t[:], in_=w1t_d[:])
        nc.sync.dma_start(out=w2t[:], in_=w2t_d[:])
        nc.sync.dma_start(out=cidx[:], in_=cidx_d[:])
        nc.sync.dma_start(out=ridx[:], in_=ridx_d[:])
        nc.vector.memset(onehot[:], 0.0)
        nc.vector.memset(onehot[:, P - 1:P], 1.0)

        # --- phase 1: MLP table (unnormalized), per-node sumsq ---
        n0 = 0
        st = 0
        while n0 < n_pad:
            w = min(ST_W, n_pad - n0)
            nb = w // H
            xt = p1.tile([P, ST_W], bf16, tag="xt", name="xt")[:, :w]
            nc.sync.dma_start(out=xt, in_=embT[:, n0:n0 + w])
            ph1 = ps1.tile([P, ST_W], f32, tag="ph1", name="ph1")[:, :w]
            nc.tensor.matmul(ph1, lhsT=w1t[:], rhs=xt, start=True, stop=True)
            # elu(x) = max(exp(min(x, 0)) - 1, x); exp(min(x,0)) = exp(-relu(-x))
            u_t = p1.tile([P, ST_W], bf16, tag="u", name="u")[:, :w]
            nc.scalar.activation(u_t, ph1, Act.Relu, scale=-1.0)
            e_t = p1.tile([P, ST_W], bf16, tag="e", name="e")[:, :w]
            nc.scalar.activation(e_t, u_t, Act.Exp, scale=-1.0)
            h1_t = p1.tile([P, ST_W], bf16, tag="h1", name="h1")[:, :w]
            nc.vector.scalar_tensor_tensor(
                h1_t, in0=e_t, scalar=-1.0, in1=ph1,
                op0=Alu.add, op1=Alu.max)
            pg = ps2.tile([P, ST_W], f32, tag="pg", name="pg")[:, :w]
            for b in range(nb):
                nc.tensor.matmul(pg[:, b * H:(b + 1) * H],
                                 lhsT=h1_t[:, b * H:(b + 1) * H],
                                 rhs=w2t[:], start=True, stop=True)
            # stage unnormalized G into the table, and accumulate sum(G^2)
            nc.scalar.activation(table[:, n0:n0 + w], pg, Act.Copy)
            sq_t = p1.tile([P, ST_W], bf16, tag="sq", name="sq")[:, :w]
            for b in range(nb):
                nc.vector.scalar_tensor_tensor(
                    sq_t[:, b * H:(b + 1) * H],
                    in0=pg[:, b * H:(b + 1) * H], scalar=0.0,
                    in1=table[:, n0 + b * H:n0 + (b + 1) * H],
                    op0=Alu.add, op1=Alu.mult,
                    accum_out=ss_all[:, st * (ST_W // H) + b:
                                     st * (ST_W // H) + b + 1])
            n0 += w
            st += 1

        # --- batched normalization factors ---
        s_all = const.tile([P, n_blocks], f32, tag="s_all")
        nc.scalar.activation(s_all[:], ss_all[:], Act.Sqrt)
        m_all = const.tile([P, n_blocks], f32, tag="m_all")
        nc.vector.tensor_scalar_max(m_all[:], s_all[:], 1e-8)
        nc.vector.reciprocal(r_all[:], m_all[:])

        # --- in-place table normalization (per 128-node block) ---
        # NOTE: in-place tensor_tensor with a step-0 broadcast in1 hard-crashes
        # the DVE on HW; per-block in-place tensor_scalar_mul works.
        for blk in range(n_blocks):
            nc.vector.tensor_scalar_mul(
                table[:, blk * H:(blk + 1) * H],
                table[:, blk * H:(blk + 1) * H],
                r_all[:, blk:blk + 1])

        # --- phase 2: gather + per-edge dot products ---
        halves = (table[:, :half], table[:, half:n_pad])

        chunk_id = 0
        gq = 0
        pout = None
        stream0 = 0
        for k in range(4):
            kc, kr = k >> 1, k & 1   # 0 -> half A, 1 -> half B
            bucket_edges = nck[k] * CHUNK
            t0 = 0
            while t0 < bucket_edges:
                tsz = min(GT, bucket_edges - t0)
                w0 = (stream0 + t0) // 16
                w1 = (stream0 + t0 + tsz) // 16
                f1t = p2.tile([P, GT], bf16, tag="f1", name="f1")
                f2t = p2.tile([P, GT], bf16, tag="f2", name="f2")
                f1g = f1t[:, :tsz].rearrange("p (a t) -> p a t", a=1)
                f2g = f2t[:, :tsz].rearrange("p (a t) -> p a t", a=1)
                nc.gpsimd.dma_gather(
                    f1g, halves[kc], cidx[:, w0:w1], tsz, tsz, H,
                    transpose=True, sbuf_tokens_per_rank=P,
                    sbuf_free_dim_per_rank=256, single_packet=False,
                    queue_num=gq % NSWQ)
                nc.gpsimd.dma_gather(
                    f2g, halves[kr], ridx[:, w0:w1], tsz, tsz, H,
                    transpose=True, sbuf_tokens_per_rank=P,
                    sbuf_free_dim_per_rank=256, single_packet=False,
                    queue_num=(gq + 1) % NSWQ)
                gq += 2
                for c in range(tsz // CHUNK):
                    prod = pprod.tile([P, CHUNK], bf16, tag="prod")
                    nc.vector.tensor_tensor(
                        out=prod[:], in0=f1t[:, c * CHUNK:(c + 1) * CHUNK],
                        in1=f2t[:, c * CHUNK:(c + 1) * CHUNK], op=Alu.mult)
                    g, p = divmod(chunk_id, P)
                    if p == 0:
                        pout = pso.tile([P, CHUNK], f32, tag="pout")
                    last = chunk_id == n_chunks - 1
                    nc.tensor.matmul(pout[:], lhsT=onehot[:, P - 1 - p:2 * P - 1 - p],
                                     rhs=prod[:], start=(p == 0),
                                     stop=(p == P - 1 or last))
                    chunk_id += 1
                    if p == P - 1 or last:
                        rows = p + 1
                        ost = p2.tile([P, CHUNK], f32, tag="ost", name="ost")[:rows]
                        nc.vector.tensor_copy(out=ost, in_=pout[:rows])
                        nc.sync.dma_start(out=out_d[g, :rows], in_=ost)
                t0 += tsz
            stream0 += bucket_edges

    nc.compile()
    return nc


def _wrap_idx(idx):
    """[S*16] int16 -> [128, S] wrapped layout (16 partitions, replicated 8x)."""
    w = idx.reshape(-1, 16).T.astype(np.int16)
    return np.tile(w, (8, 1))


def _ensure_ntff_hook():
    """Provide antenv.axon_hooks if the image lacks it (trace support only)."""
    import sys
    import types
    try:
        import antenv.axon_hooks  # noqa: F401
        return
    except ImportError:
        pass
    try:
        import antenv
        from trn_agent_boot.trn_boot import _ntff_profile_via_ctypes
        mod = types.ModuleType("antenv.axon_hooks")
        mod._hook = _ntff_profile_via_ctypes("/opt/axon/libaxon_pjrt.so")
        mod.get_axon_ntff_profile_hook = lambda: mod._hook
        mod.set_axon_ntff_profile_hook = lambda h: setattr(mod, "_hook", h)
        sys.modules["antenv.axon_hooks"] = mod
        antenv.axon_hooks = mod
    except Exception:
        pass


def kernel(emb, edge_index, W1, b1, W2, b2):
    global LAST_RESULTS
    from concourse.bass_utils import run_bass_kernel_spmd
    _ensure_ntff_hook()

    emb = np.asarray(emb, dtype=np.float32)
    W1 = np.asarray(W1, dtype=np.float32)
    W2 = np.asarray(W2, dtype=np.float32)
    b1 = np.asarray(b1, dtype=np.float32)
    b2 = np.asarray(b2, dtype=np.float32)
    assert np.abs(b1).max() == 0 and np.abs(b2).max() == 0, \
        "nonzero biases not implemented"
    col = np.asarray(edge_index[0]).astype(np.int64)
    row = np.asarray(edge_index[1]).astype(np.int64)

    n, h = emb.shape
    assert h == H
    E = col.shape[0]
    assert E % NCORES == 0
    ec = E // NCORES
    n_pad = ((n + P - 1) // P) * P
    half = min(HALF, n_pad)

    # ---- host prep: per-core bucketed edge streams ----
    cores = []
    for ci in range(NCORES):
        c = col[ci * ec:(ci + 1) * ec]
        r = row[ci * ec:(ci + 1) * ec]
        b = ((c >= half).astype(np.int8) << 1) | (r >= half).astype(np.int8)
        order = np.argsort(b, kind="stable")
        counts = np.bincount(b, minlength=4)
        cores.append((c[order], r[order], order, counts))

    nck = tuple(int(math.ceil(max(cr[3][k] for cr in cores) / CHUNK))
                for k in range(4))
    n_chunks = sum(nck)
    n_groups = math.ceil(n_chunks / P)

    key = (n_pad, half, nck)
    if key not in _PROG_CACHE:
        _PROG_CACHE[key] = _build_program(n_pad, half, nck)
    nc = _PROG_CACHE[key]

    # ---- per-core input maps ----
    embT = np.zeros((P, n_pad), dtype=BF16)
    embT[:, :n] = emb.T.astype(BF16)
    w1t = W1.T.astype(BF16)
    w2t = W2.T.astype(BF16)

    in_maps = []
    for (cs, rs, order, counts) in cores:
        c_stream = np.zeros(n_chunks * CHUNK, dtype=np.int64)
        r_stream = np.zeros(n_chunks * CHUNK, dtype=np.int64)
        off_e = 0   # offset into sorted edge arrays
        off_s = 0   # offset into padded stream
        for k in range(4):
            cnt = counts[k]
            sel = slice(off_e, off_e + cnt)
            c_stream[off_s:off_s + cnt] = cs[sel] - (half if k >> 1 else 0)
            r_stream[off_s:off_s + cnt] = rs[sel] - (half if k & 1 else 0)
            off_e += cnt
            off_s += nck[k] * CHUNK
        in_maps.append({
            "embT": embT, "w1t": w1t, "w2t": w2t,
            "cidx": _wrap_idx(c_stream), "ridx": _wrap_idx(r_stream),
        })

    res = run_bass_kernel_spmd(nc, in_maps, core_ids=list(range(NCORES)))
    LAST_RESULTS = res

    # ---- reassemble ----
    out = np.empty(E, dtype=np.float32)
    for ci, (cs, rs, order, counts) in enumerate(cores):
        stream = res.results[ci]["out"].reshape(-1)   # chunk-major dots
        vals = np.empty(ec, dtype=np.float32)
        off_e = 0
        off_s = 0
        for k in range(4):
            cnt = counts[k]
            vals[off_e:off_e + cnt] = stream[off_s * CHUNK:off_s * CHUNK + cnt]
            off_e += cnt
            off_s += nck[k]
        seg = out[ci * ec:(ci + 1) * ec]
        seg[order] = vals
    return out



# revision 7
# speedup vs baseline: 1.6282x; 1.6282x over previous
"""Trainium2 Bass kernel for nn_ExtractorMLP (GNN edge cosine-similarity).

Math:  out[e] = cos_sim(mlp(emb[col[e]]), mlp(emb[row[e]]))
where  mlp(x) = elu(x @ W1.T + b1) @ W2.T + b2   (b1 = b2 = 0 here)

mlp is row-wise so mlp(emb[idx]) == mlp(emb)[idx]: compute the normalized
MLP table t[v] = g[v]/max(||g[v]||,eps) once per core (phase 1), then per
edge out[e] = dot(t[col[e]], t[row[e]]) (phase 2).

Phase 2 (per-core edge slice, sharded 8 ways).  Key change vs v1: only ONE
side (row) uses the GPSIMD dma_gather -- its Q7 descriptor-generation
ucode (~8ns/row, serialized on the Pool engine) was 84% of v1's runtime.
The col side is recovered on the (otherwise idle) tensor engine:

  * Edges are sorted by (row_half, col).  Within a 128-edge chunk the
    sorted cols span only a few aligned 128-node blocks of the table.
  * col side: one-hot expansion.  onehot_j[v,e] = (colRel[e] == v + 128j)
    built on DVE (iota constants supplied from host), then
    psum_f1[f,e] += table_block(b0+j) @ onehot_j on the tensor engine
    -> f1T [feat, edge] in PSUM.  Per-chunk (b0, span) metadata is merged
    across the 8 cores (min-b0 / max-hi) so the SPMD program is shared.
  * row side: SBUF-source transposed dma_gather (int16 indices; table
    halves at 32768 nodes; the (row_half, col) sort gives each bucket one
    half) -> f2T [feat, edge] in SBUF.
  * dot: DVE multiply, then contraction over features with a sliding
    one-hot matmul packing supertile s's 512 dots into partition s%128 of
    PSUM output group s//128.

ELU identity used on device:  elu(x) = max(exp(-relu(-x)) - 1, x)
"""

import math

import numpy as np
import ml_dtypes

BF16 = ml_dtypes.bfloat16

H = 128          # feature dim
P = 128          # partitions
ST = 512         # edges per supertile (psum/output granularity)
CH = 128         # edges per col-expansion chunk
GT = 4096        # edges per dma_gather instruction
HALF = 32768     # int16 index limit: table split point
NCORES = 8
NSWQ = 1         # SWDGE queues: >1 corrupts concurrent gathers (racy ucode)
ST_W = 512       # phase-1 supertile width (nodes)
MAXSPAN = 8      # iota pieces provisioned (merged chunk spans stay small)

_PROG_CACHE: dict = {}
LAST_RESULTS = None  # test harness can inspect exec_time_ns


def _build_program(n_pad, half, nck, chunk_meta, num_devices=NCORES):
    """Build the (shared, SPMD) bass program.

    n_pad: padded node count (multiple of 128)
    half:  table split element offset
    nck:   per-bucket supertile counts (len 2), shared across cores
    chunk_meta: tuple over all chunks (bucket 0 then bucket 1) of
        (b0, span): merged first block / block span of the chunk's cols.
    """
    import concourse.bacc as bacc
    import concourse.mybir as mybir
    import concourse.tile as tile
    from concourse import library_config
    from contextlib import ExitStack

    f32 = mybir.dt.float32
    bf16 = mybir.dt.bfloat16
    i16 = mybir.dt.int16
    Alu = mybir.AluOpType
    Act = mybir.ActivationFunctionType

    n_st = sum(nck)                      # total supertiles
    n_groups = math.ceil(n_st / P)
    stream_cols = n_st * (ST // 16)      # wrapped row-idx columns
    n_blocks = n_pad // H

    nc = bacc.Bacc("TRN2", target_bir_lowering=False, debug=False,
                   num_devices=num_devices, num_swdge_queues=NSWQ)

    embT = nc.dram_tensor("embT", [P, n_pad], bf16, kind="ExternalInput")
    w1t_d = nc.dram_tensor("w1t", [H, H], bf16, kind="ExternalInput")
    w2t_d = nc.dram_tensor("w2t", [H, H], bf16, kind="ExternalInput")
    ridx_d = nc.dram_tensor("ridx", [P, stream_cols], i16, kind="ExternalInput")
    crel_d = nc.dram_tensor("crel", [1, n_st * ST], i16, kind="ExternalInput")
    iota_d = nc.dram_tensor("iota", [P, MAXSPAN * ST], i16, kind="ExternalInput")
    out_d = nc.dram_tensor("out", [n_groups, P, ST], f32, kind="ExternalOutput")

    with ExitStack() as ctx:
        tc = ctx.enter_context(tile.TileContext(nc))
        const = ctx.enter_context(tc.tile_pool(name="const", bufs=1))
        p1 = ctx.enter_context(tc.tile_pool(name="p1", bufs=3))
        pf2 = ctx.enter_context(tc.tile_pool(name="pf2", bufs=2))
        pcr = ctx.enter_context(tc.tile_pool(name="pcr", bufs=2))
        poh = ctx.enter_context(tc.tile_pool(name="poh", bufs=MAXSPAN + 4))
        pprod = ctx.enter_context(tc.tile_pool(name="pprod", bufs=4))
        pout_sb = ctx.enter_context(tc.tile_pool(name="pout_sb", bufs=2))
        ps1 = ctx.enter_context(tc.tile_pool(name="ps1", bufs=2, space="PSUM"))
        ps2 = ctx.enter_context(tc.tile_pool(name="ps2", bufs=2, space="PSUM"))
        psf1 = ctx.enter_context(tc.tile_pool(name="psf1", bufs=2, space="PSUM"))
        pso = ctx.enter_context(tc.tile_pool(name="pso", bufs=2, space="PSUM"))

        nc.gpsimd.load_library(library_config.mlp)

        # --- constants / persistent tiles ---
        table = const.tile([P, n_pad], bf16, tag="table")
        w1t = const.tile([H, H], bf16, tag="w1t")
        w2t = const.tile([H, H], bf16, tag="w2t")
        # sliding one-hot: onehot[:, 127-p : 255-p] has ones in column p only;
        # used as lhsT so supertile p's dot-row lands in PSUM partition p.
        onehot = const.tile([P, 2 * P - 1], bf16, tag="onehot")
        ss_all = const.tile([P, n_blocks], f32, tag="ss_all")
        r_all = const.tile([P, n_blocks], f32, tag="r_all")
        ridx = const.tile([P, stream_cols], i16, tag="ridx")
        iotas = const.tile([P, MAXSPAN * ST], i16, tag="iotas")
        nc.sync.dma_start(out=w1t[:], in_=w1t_d[:])
        nc.sync.dma_start(out=w2t[:], in_=w2t_d[:])
        nc.sync.dma_start(out=ridx[:], in_=ridx_d[:])
        nc.sync.dma_start(out=iotas[:], in_=iota_d[:])
        nc.vector.memset(onehot[:], 0.0)
        nc.vector.memset(onehot[:, P - 1:P], 1.0)

        # --- phase 1: MLP table (unnormalized), per-node sumsq ---
        n0 = 0
        sti = 0
        while n0 < n_pad:
            w = min(ST_W, n_pad - n0)
            nb = w // H
            xt = p1.tile([P, ST_W], bf16, tag="xt", name="xt")[:, :w]
            nc.sync.dma_start(out=xt, in_=embT[:, n0:n0 + w])
            ph1 = ps1.tile([P, ST_W], f32, tag="ph1", name="ph1")[:, :w]
            nc.tensor.matmul(ph1, lhsT=w1t[:], rhs=xt, start=True, stop=True)
            # elu(x) = max(exp(min(x, 0)) - 1, x); exp(min(x,0)) = exp(-relu(-x))
            u_t = p1.tile([P, ST_W], bf16, tag="u", name="u")[:, :w]
            nc.scalar.activation(u_t, ph1, Act.Relu, scale=-1.0)
            e_t = p1.tile([P, ST_W], bf16, tag="e", name="e")[:, :w]
            nc.scalar.activation(e_t, u_t, Act.Exp, scale=-1.0)
            h1_t = p1.tile([P, ST_W], bf16, tag="h1", name="h1")[:, :w]
            nc.vector.scalar_tensor_tensor(
                h1_t, in0=e_t, scalar=-1.0, in1=ph1,
                op0=Alu.add, op1=Alu.max)
            pg = ps2.tile([P, ST_W], f32, tag="pg", name="pg")[:, :w]
            for b in range(nb):
                nc.tensor.matmul(pg[:, b * H:(b + 1) * H],
                                 lhsT=h1_t[:, b * H:(b + 1) * H],
                                 rhs=w2t[:], start=True, stop=True)
            # stage unnormalized G into the table, and accumulate sum(G^2)
            nc.scalar.activation(table[:, n0:n0 + w], pg, Act.Copy)
            sq_t = p1.tile([P, ST_W], bf16, tag="sq", name="sq")[:, :w]
            for b in range(nb):
                nc.vector.scalar_tensor_tensor(
                    sq_t[:, b * H:(b + 1) * H],
                    in0=pg[:, b * H:(b + 1) * H], scalar=0.0,
                    in1=table[:, n0 + b * H:n0 + (b + 1) * H],
                    op0=Alu.add, op1=Alu.mult,
                    accum_out=ss_all[:, sti * (ST_W // H) + b:
                                     sti * (ST_W // H) + b + 1])
            n0 += w
            sti += 1

        # --- batched normalization factors ---
        s_all = const.tile([P, n_blocks], f32, tag="s_all")
        nc.scalar.activation(s_all[:], ss_all[:], Act.Sqrt)
        m_all = const.tile([P, n_blocks], f32, tag="m_all")
        nc.vector.tensor_scalar_max(m_all[:], s_all[:], 1e-8)
        nc.vector.reciprocal(r_all[:], m_all[:])

        # --- in-place table normalization (per 128-node block) ---
        # NOTE: in-place tensor_tensor with a step-0 broadcast in1 hard-crashes
        # the DVE on HW; per-block in-place tensor_scalar_mul works.
        for blk in range(n_blocks):
            nc.vector.tensor_scalar_mul(
                table[:, blk * H:(blk + 1) * H],
                table[:, blk * H:(blk + 1) * H],
                r_all[:, blk:blk + 1])

        # --- phase 2 ---
        halves = (table[:, :half], table[:, half:n_pad])

        pout = None
        for hb in range(2):
            st_base = nck[0] * (ST // 16) * 0 + (nck[0] if hb else 0)
            bucket_edges = nck[hb] * ST
            t0 = 0          # edge offset within bucket
            while t0 < bucket_edges:
                tsz = min(GT, bucket_edges - t0)
                e0 = (st_base * ST) + t0      # global edge offset
                f2t = pf2.tile([P, GT], bf16, tag="f2", name="f2")
                f2g = f2t[:, :tsz].rearrange("p (a t) -> p a t", a=1)
                nc.gpsimd.dma_gather(
                    f2g, halves[hb], ridx[:, e0 // 16:(e0 + tsz) // 16],
                    tsz, tsz, H,
                    transpose=True, sbuf_tokens_per_rank=P,
                    sbuf_free_dim_per_rank=256, single_packet=False,
                    queue_num=0)
                crt = pcr.tile([P, GT], i16, tag="crt", name="crt")
                # broadcast-replicate the [1, tsz] colRel row across all 128
                # partitions during the DMA itself (step-0 DRAM source)
                nc.sync.dma_start(
                    out=crt[:, :tsz],
                    in_=crel_d[:, e0:e0 + tsz].to_broadcast([P, tsz]))

                for s in range(tsz // ST):
                    stg = st_base + t0 // ST + s      # global supertile id
                    sl = slice(s * ST, (s + 1) * ST)
                    # one-hots for every piece-j used by this supertile
                    used = set()
                    for ci in range(ST // CH):
                        _, span = chunk_meta[stg * (ST // CH) + ci]
                        used.update(range(span))
                    ohs = {}
                    for j in sorted(used):
                        oh = poh.tile([P, ST], bf16, tag="oh", name=f"oh{j}")
                        nc.vector.tensor_tensor(
                            out=oh[:],
                            in0=crt[:, sl],
                            in1=iotas[:, j * ST:(j + 1) * ST],
                            op=Alu.is_equal)
                        ohs[j] = oh
                    # col-side expansion into PSUM f1T [feat, edge]
                    f1p = psf1.tile([P, ST], f32, tag="f1p", name="f1p")
                    for ci in range(ST // CH):
                        b0, span = chunk_meta[stg * (ST // CH) + ci]
                        csl = slice(ci * CH, (ci + 1) * CH)
                        for j in range(span):
                            blk = b0 + j
                            nc.tensor.matmul(
                                f1p[:, csl],
                                lhsT=table[:, blk * H:(blk + 1) * H],
                                rhs=ohs[j][:, csl],
                                start=(j == 0), stop=(j == span - 1))
                    # dot: multiply then pack supertile dots via sliding onehot
                    prod = pprod.tile([P, ST], bf16, tag="prod", name="prod")
                    nc.vector.tensor_tensor(
                        out=prod[:], in0=f1p[:], in1=f2t[:, sl],
                        op=Alu.mult)
                    g, p = divmod(stg, P)
                    if p == 0:
                        pout = pso.tile([P, ST], f32, tag="pout")
                    last = stg == n_st - 1
                    nc.tensor.matmul(pout[:],
                                     lhsT=onehot[:, P - 1 - p:2 * P - 1 - p],
                                     rhs=prod[:], start=(p == 0),
                                     stop=(p == P - 1 or last))
                    if p == P - 1 or last:
                        rows = p + 1
                        ost = pout_sb.tile([P, ST], f32, tag="ost",
                                           name="ost")[:rows]
                        nc.vector.tensor_copy(out=ost, in_=pout[:rows])
                        nc.sync.dma_start(out=out_d[g, :rows], in_=ost)
                t0 += tsz

    nc.compile()
    return nc


def _wrap_idx(idx):
    """[S*16] int16 -> [128, S] wrapped layout (16 partitions, replicated 8x)."""
    w = idx.reshape(-1, 16).T.astype(np.int16)
    return np.tile(w, (8, 1))


def _ensure_ntff_hook():
    """Provide antenv.axon_hooks if the image lacks it (trace support only)."""
    import sys
    import types
    try:
        import antenv.axon_hooks  # noqa: F401
        return
    except ImportError:
        pass
    try:
        import antenv
        from trn_agent_boot.trn_boot import _ntff_profile_via_ctypes
        mod = types.ModuleType("antenv.axon_hooks")
        mod._hook = _ntff_profile_via_ctypes("/opt/axon/libaxon_pjrt.so")
        mod.get_axon_ntff_profile_hook = lambda: mod._hook
        mod.set_axon_ntff_profile_hook = lambda h: setattr(mod, "_hook", h)
        sys.modules["antenv.axon_hooks"] = mod
        antenv.axon_hooks = mod
    except Exception:
        pass


def _prep_core(c, r, half):
    """Sort one core's edges by (row_half, col).

    Returns (order, n_real0, n_real1, cols_sorted, rowlocal_sorted) with
    cols/rows NOT yet padded (bucket sizes = real counts).
    """
    hb = (r >= half).astype(np.int8)
    order = np.lexsort((c, hb))          # row_half major, col minor
    cs, rs, hs = c[order], r[order], hb[order]
    n0 = int((hs == 0).sum())
    return order, n0, cs, rs - hb[order].astype(rs.dtype) * half


def kernel(emb, edge_index, W1, b1, W2, b2):
    global LAST_RESULTS
    from concourse.bass_utils import run_bass_kernel_spmd
    _ensure_ntff_hook()

    emb = np.asarray(emb, dtype=np.float32)
    W1 = np.asarray(W1, dtype=np.float32)
    W2 = np.asarray(W2, dtype=np.float32)
    b1 = np.asarray(b1, dtype=np.float32)
    b2 = np.asarray(b2, dtype=np.float32)
    assert np.abs(b1).max() == 0 and np.abs(b2).max() == 0, \
        "nonzero biases not implemented"
    col = np.asarray(edge_index[0]).astype(np.int64)
    row = np.asarray(edge_index[1]).astype(np.int64)

    n, h = emb.shape
    assert h == H
    E = col.shape[0]
    assert E % NCORES == 0
    ec = E // NCORES
    n_pad = ((n + P - 1) // P) * P
    half = min(HALF, n_pad)
    n_blocks = n_pad // H

    cores = [
        _prep_core(col[ci * ec:(ci + 1) * ec], row[ci * ec:(ci + 1) * ec],
                   half)
        for ci in range(NCORES)
    ]

    # shared bucket supertile counts
    nck = tuple(
        max(math.ceil(((cr[1]) if hb == 0 else (ec - cr[1])) / ST)
            for cr in cores)
        for hb in range(2)
    )
    n_st = sum(nck)
    n_chunks = n_st * (ST // CH)
    n_groups = math.ceil(n_st / P)

    # per-core padded streams + per-chunk (b0, hi) for merging
    padded = []           # (cols_padded, rowl_padded) per core
    b0s = np.empty((NCORES, n_chunks), dtype=np.int64)
    his = np.empty((NCORES, n_chunks), dtype=np.int64)
    for ci, (order, n0, cs, rl) in enumerate(cores):
        segs_c, segs_r = [], []
        for hb in range(2):
            bc = cs[:n0] if hb == 0 else cs[n0:]
            br = rl[:n0] if hb == 0 else rl[n0:]
            tgt = nck[hb] * ST
            pad = tgt - len(bc)
            assert pad >= 0
            fillc = bc[-1] if len(bc) else 0
            segs_c.append(np.concatenate(
                [bc, np.full(pad, fillc, dtype=cs.dtype)]))
            segs_r.append(np.concatenate(
                [br, np.zeros(pad, dtype=rl.dtype)]))
        cp = np.concatenate(segs_c)
        rp = np.concatenate(segs_r)
        padded.append((cp, rp))
        cc = cp.reshape(n_chunks, CH)
        b0s[ci] = cc[:, 0] >> 7
        his[ci] = cc[:, -1] >> 7

    b0m = b0s.min(axis=0)
    him = his.max(axis=0)
    spans = him - b0m + 1
    assert spans.max() <= MAXSPAN, f"merged span {spans.max()} > {MAXSPAN}"
    chunk_meta = tuple(zip(b0m.tolist(), spans.tolist()))

    key = (n_pad, half, nck, chunk_meta)
    if key not in _PROG_CACHE:
        _PROG_CACHE[key] = _build_program(n_pad, half, nck, chunk_meta)
    nc = _PROG_CACHE[key]

    # ---- shared constant inputs ----
    embT = np.zeros((P, n_pad), dtype=BF16)
    embT[:, :n] = emb.T.astype(BF16)
    w1t = W1.T.astype(BF16)
    w2t = W2.T.astype(BF16)
    iota = np.empty((P, MAXSPAN * ST), dtype=np.int16)
    base = np.arange(P, dtype=np.int16)[:, None]
    for j in range(MAXSPAN):
        iota[:, j * ST:(j + 1) * ST] = base + j * CH

    in_maps = []
    for ci in range(NCORES):
        cp, rp = padded[ci]
        crel = (cp.reshape(n_chunks, CH)
                - (b0m[:, None] << 7)).astype(np.int16).reshape(1, -1)
        assert crel.min() >= 0 and crel.max() < MAXSPAN * CH
        in_maps.append({
            "embT": embT, "w1t": w1t, "w2t": w2t,
            "ridx": _wrap_idx(rp.astype(np.int16)),
            "crel": crel, "iota": iota,
        })

    res = run_bass_kernel_spmd(nc, in_maps, core_ids=list(range(NCORES)))
    LAST_RESULTS = res

    # ---- reassemble ----
    out = np.empty(E, dtype=np.float32)
    for ci, (order, n0, cs, rl) in enumerate(cores):
        stream = res.results[ci]["out"].reshape(n_groups * P, ST)
        vals = np.empty(ec, dtype=np.float32)
        vals[:n0] = stream.reshape(-1)[:n0]
        off1 = nck[0] * ST
        vals[n0:] = stream.reshape(-1)[off1:off1 + (ec - n0)]
        seg = out[ci * ec:(ci + 1) * ec]
        seg[order] = vals
    return out
